# revision 1
# baseline (speedup 1.0000x reference)
"""Trainium2 Bass kernel for nn_MeshTransformer (8-core SPMD, V-sharded).

Computes, for each of BS=256 (b,s) pairs:
    out[bs, v, i] = sum_{p,j} ws[bs,p] * R[i,j](bs,p) * deformed[p,v,j]
                    + sum_p w[bs,p] * t[bs,p,i]
with R the XYZ-euler rotation, ws = w * scale, deformed = base + offsets.

Mapping:
  - Vertex dim V (2562, padded to 2576) is sharded 8 ways (322/core).
  - Each core computes all 256 weight matrices on-chip and contracts them
    against its deformed slice on the PE (fp16 matmuls, fp32 PSUM).
  - The host ships six 256-col angle blocks, each range-folded to [-pi, pi)
    (Sin spline domain) and pre-shifted so that ONE ACT Sin op yields every
    needed trig operand, including the stacked/negated forms, as views:
      S = sin(ang6) = [ sa | ca | (sc;cc) | (cc;sc) | (sb;-sb) | (cb;-cb) ]
    (cos(x) = sin(pi/2 - x); the two 64-partition halves of a block hold
    different shifts, matching the lhsT partition packing below.)
  - lhsT partition layout packs rotation column j in 64-partition blocks,
    paired with a stacked rhs:
      LA_i = [W_i0 (p 0..63) ; W_i1 (p 64..127)]   DA = [deformed_0 ; deformed_1]
      LB_i = [W_i2          ; wt_i            ]   DB = [deformed_2 ; ones     ]
    (the ones block folds the translation term into the same contraction),
    so each rotation-row build is a single full-lane DVE op:
      LA0 = WS*(CBX*UX), LA1 = WCA*U + WSA*V, LA2 = WSA*U - WCA*V, V = SBX*UX
  - PSUM groups are accumulated over 2-3 passes (folding the la1/la2 row
    sums into the PE), drained to two fp16 [128, 3*VC] half-batch tiles, and
    DMA'd out; the host gather transposes to the reference [BS, V, 3] layout.
  - Translations ride the otherwise-dead rows of the deformed-matrix DMA.
"""

import numpy as np
from contextlib import ExitStack

import concourse.bass as bass
import concourse.tile as tile
from concourse import bacc, mybir
from concourse.bass_utils import run_bass_kernel_spmd

B, S, P, V = 16, 16, 64, 2562
BS = B * S              # 256
N_CORES = 8
VPAD = 2576             # multiple of 8; per-core N kept even
VC = VPAD // N_CORES    # 322 vertices per core

F32 = mybir.dt.float32
F16 = mybir.dt.float16
AF = mybir.ActivationFunctionType
ALU = mybir.AluOpType


def _build_kernel():
    nc = bacc.Bacc("TRN2", target_bir_lowering=False, debug=False)

    ang_d = nc.dram_tensor("ang6", [128, 1536], F16, kind="ExternalInput").ap()
    wst_d = nc.dram_tensor("wst", [128, 512], F16, kind="ExternalInput").ap()
    # offtA | bsetA | offtB/bsetB (rows 0:64)
    dmat_d = nc.dram_tensor("dmat", [128, 2 * VC + 768], F16, kind="ExternalInput").ap()
    out_d = nc.dram_tensor("out", [2, 128, 3 * VC], F16,
                           kind="ExternalOutput").ap()

    lo = slice(0, 64)
    hi = slice(64, 128)

    with tile.TileContext(nc) as tc, ExitStack() as ctx:
        pool = ctx.enter_context(tc.tile_pool(name="work", bufs=1))
        psum = ctx.enter_context(tc.tile_pool(name="psum", bufs=6, space="PSUM"))

        # preload the ACT Sin table set while the inputs are still in flight
        dummy = pool.tile([128, 1], F16, tag="dummy")
        dummy2 = pool.tile([128, 1], F16, tag="dummy2")
        nc.vector.memset(dummy[:], 0.25)
        nc.scalar.activation(dummy2[:], dummy[:], AF.Sin)

        # ---- input tiles ----
        ang = pool.tile([128, 1536], F16, tag="ang")
        wst = pool.tile([128, 512], F16, tag="wst")
        dmat = pool.tile([128, 2 * VC + 768], F16, tag="dmat")
        nc.sync.dma_start(out=ang[:], in_=ang_d[:])
        nc.sync.dma_start(out=dmat[:], in_=dmat_d[:])
        nc.sync.dma_start(out=wst[:], in_=wst_d[:])
        wraw = wst[:, 0:BS]
        scl = wst[:, BS:2 * BS]
        dta = dmat[:, 0:2 * VC]                  # offtA | bsetA
        dtb = dmat[0:64, 2 * VC:4 * VC]          # offtB | bsetB (rows 0:64)
        trn = dmat[64:128, 2 * VC:2 * VC + 768]  # translations (rows 64:128)

        # ---- deformed (rhs) ----
        da = pool.tile([128, VC], F16, tag="da")
        db = pool.tile([128, VC], F16, tag="db")
        nc.vector.memset(db[64:128, :], 1.0)         # translation ones block

        # ---- trig: one Sin over all pre-folded blocks ----
        sall = pool.tile([128, 1536], F16, tag="sall")
        nc.scalar.activation(sall[:], ang[:], AF.Sin)
        sa = sall[:, 0:256]
        ca = sall[:, 256:512]
        u = sall[:, 512:768]        # [sc ; cc]
        ux = sall[:, 768:1024]      # [cc ; sc]
        sbx = sall[:, 1024:1280]    # [sb ; -sb]
        cbx = sall[:, 1280:1536]    # [cb ; -cb]

        # ---- weight products (fp16, full-lane) ----
        ws = pool.tile([128, BS], F16, tag="ws")
        wca = pool.tile([128, BS], F16, tag="wca")
        wsa = pool.tile([128, BS], F16, tag="wsa")
        p1 = pool.tile([128, BS], F16, tag="p1")      # [cbcc ; -cbsc]
        v = pool.tile([128, BS], F16, tag="v")        # [sbcc ; -sbsc]
        la0 = pool.tile([128, BS], F16, tag="la0")
        lb = [pool.tile([128, BS], F16, name=f"lb{i}", tag=f"lb{i}") for i in range(3)]
        ta = pool.tile([128, BS], F16, tag="ta")
        tb = pool.tile([128, BS], F16, tag="tb")
        tc_ = pool.tile([128, BS], F16, tag="tc_")
        td = pool.tile([128, BS], F16, tag="td")

        # translation weights: no trig dependency, run during the Sin op
        nc.gpsimd.tensor_mul(lb[0][hi, :], wraw[hi, :], trn[:, 0:BS])
        nc.gpsimd.tensor_mul(lb[1][hi, :], wraw[hi, :], trn[:, BS:2 * BS])
        nc.gpsimd.tensor_mul(lb[2][hi, :], wraw[hi, :], trn[:, 2 * BS:3 * BS])

        nc.vector.tensor_mul(ws[:], wraw, scl)

        # i=0 row first so PE can start early
        nc.vector.tensor_mul(p1[:], cbx, ux)
        nc.vector.tensor_mul(la0[:], ws[:], p1[:])
        nc.vector.tensor_add(da[:], dta[:, 0:VC], dta[:, VC:2 * VC])
        nc.vector.tensor_mul(wca[:], ws[:], ca)
        nc.vector.tensor_mul(wsa[:], ws[:], sa)
        nc.vector.tensor_mul(v[:], sbx, ux)
        nc.vector.tensor_mul(lb[0][lo, :], ws[lo, :], sbx[lo, :])
        nc.vector.tensor_add(db[0:64, :], dtb[:, 0:VC], dtb[:, VC:2 * VC])

        # i=1 row: la1 = ta + tb is folded into PSUM accumulation
        nc.vector.tensor_mul(ta[:], wca[:], u)
        nc.vector.tensor_mul(tb[:], wsa[:], v[:])
        nc.vector.scalar_tensor_tensor(
            lb[1][lo, :], cbx[lo, :], -1.0, wsa[lo, :], op0=ALU.mult, op1=ALU.mult)

        # i=2 row: la2 = tc - td via PSUM accumulation with negated v
        vneg = pool.tile([128, BS], F16, tag="vneg")
        nc.vector.tensor_scalar_mul(vneg[:], v[:], -1.0)
        nc.vector.tensor_mul(tc_[:], wsa[:], u)
        nc.vector.tensor_mul(td[:], wca[:], vneg[:])
        nc.vector.tensor_mul(lb[2][lo, :], wca[lo, :], cbx[lo, :])

        # ---- matmuls (PSUM-accumulated row sums) + drain + output ----
        osb2 = [pool.tile([128, 3 * VC], F16, name=f"osbh{h}", tag=f"osbh{h}")
                for h in range(2)]
        pss = {}
        passes = {0: [(la0, da), (lb[0], db)],
                  1: [(ta, da), (tb, da), (lb[1], db)],
                  2: [(tc_, da), (td, da), (lb[2], db)]}
        for i, h in [(0, 0), (0, 1), (1, 0), (1, 1), (2, 0), (2, 1)]:
            ms = slice(h * 128, (h + 1) * 128)
            ps = psum.tile([128, VC], F32)
            plist = passes[i]
            for k, (lt, rt) in enumerate(plist):
                nc.tensor.matmul(ps[:], lt[:, ms], rt[:],
                                 start=(k == 0), stop=(k == len(plist) - 1))
            pss[(i, h)] = ps
        # drain (i0,i1) groups first so their DMAs fly before i2 lands
        nc.scalar.copy(osb2[0][:, 0:VC], pss[(0, 0)][:])
        nc.scalar.copy(osb2[0][:, VC:2 * VC], pss[(1, 0)][:])
        nc.scalar.copy(osb2[1][:, 0:VC], pss[(0, 1)][:])
        nc.scalar.copy(osb2[1][:, VC:2 * VC], pss[(1, 1)][:])
        nc.vector.tensor_copy(osb2[0][:, 2 * VC:3 * VC], pss[(2, 0)][:])
        nc.scalar.copy(osb2[1][:, 2 * VC:3 * VC], pss[(2, 1)][:])
        for h in range(2):
            nc.sync.dma_start(out=out_d[h], in_=osb2[h][:])

    nc.compile()
    return nc


_NC_CACHE = None


def _get_nc():
    global _NC_CACHE
    if _NC_CACHE is None:
        _NC_CACHE = _build_kernel()
    return _NC_CACHE


def _fold(x):
    """Range-fold to [-pi, pi) (Sin spline domain)."""
    return np.mod(x + np.pi, 2 * np.pi) - np.pi


def _prep_inputs(scales, transforms, prototype_weights, prototype_offsets, base_verts):
    """Host-side shard/layout prep (layout, dup, angle folding/shifting)."""
    f = np.float64
    hh = np.float16
    scl1 = np.asarray(scales, np.float32).reshape(BS)
    tf = np.asarray(transforms, np.float32).reshape(BS, P, 6)

    a = tf[:, :, 3].T.astype(f)   # [p, bs]
    b = tf[:, :, 4].T.astype(f)
    c = tf[:, :, 5].T.astype(f)
    P2 = np.pi / 2

    def blk(lov, hiv):
        return np.concatenate([_fold(lov), _fold(hiv)], axis=0)   # [128, bs]

    ang6 = np.concatenate([
        blk(a, a),              # sa
        blk(P2 - a, P2 - a),    # ca
        blk(c, P2 - c),         # [sc ; cc]
        blk(P2 - c, c),         # [cc ; sc]
        blk(b, -b),             # [sb ; -sb]
        blk(P2 - b, b - P2),    # [cb ; -cb]
    ], axis=1).astype(hh)                                         # [128, 1536]

    w_h = np.asarray(prototype_weights, np.float32).reshape(BS, P).T
    wraw = np.concatenate([w_h, w_h], axis=0)                     # [128, 256]
    scl = np.broadcast_to(scl1[None, :], (128, BS))
    trn_h = tf[:, :, 0:3].transpose(1, 2, 0).reshape(P, 3 * BS)   # [64, 768]

    offp = np.zeros((P, VPAD, 3), np.float32)
    offp[:, :V] = np.asarray(prototype_offsets, np.float32)
    offt = offp.transpose(2, 0, 1).reshape(192, VPAD)
    basep = np.zeros((VPAD, 3), np.float32)
    basep[:V] = np.asarray(base_verts, np.float32)
    bset = np.broadcast_to(basep.T[:, None, :], (3, P, VPAD)).reshape(192, VPAD)

    in_maps = []
    for core in range(N_CORES):
        vs = slice(core * VC, (core + 1) * VC)
        oA, bA = offt[0:128, vs], bset[0:128, vs]
        oB, bB = offt[128:192, vs], bset[128:192, vs]
        dB = np.zeros((128, 768), np.float32)
        dB[0:64, 0:VC] = oB
        dB[0:64, VC:2 * VC] = bB
        dB[64:128, 0:768] = trn_h          # translations ride dtb's dead rows
        wst = np.concatenate([wraw, scl], axis=1)
        dmat = np.concatenate([oA, bA, dB], axis=1)
        in_maps.append({"ang6": ang6, "wst": wst.astype(hh),
                        "dmat": dmat.astype(hh)})
    return in_maps


def kernel(scales, transforms, prototype_weights, prototype_offsets, base_verts):
    nc = _get_nc()
    in_maps = _prep_inputs(
        scales, transforms, prototype_weights, prototype_offsets, base_verts)
    res = run_bass_kernel_spmd(nc, in_maps, list(range(N_CORES)))
    full = np.empty((BS, VPAD, 3), np.float32)
    for c in range(N_CORES):
        planes = res.results[c]["out"].astype(np.float32)
        vs = slice(c * VC, (c + 1) * VC)
        for i in range(3):
            for h in range(2):
                full[h * 128:(h + 1) * 128, vs, i] = \
                    planes[h][:, i * VC:(i + 1) * VC]
    return np.ascontiguousarray(full[:, :V, :])



# revision 5
# speedup vs baseline: 1.2655x; 1.2655x over previous
"""Trainium2 Bass kernel for nn_MeshTransformer (8-core SPMD, V-sharded).

Computes, for each of BS=256 (b,s) pairs:
    out[bs, v, i] = sum_{p,j} ws[bs,p] * R[i,j](bs,p) * deformed[p,v,j]
                    + sum_p w[bs,p] * t[bs,p,i]
with R the XYZ-euler rotation, ws = w * scale, deformed = base + offsets.

Mapping:
  - Vertex dim V (2562, padded to 2576) is sharded 8 ways (322/core).
  - The host precomputes every weight product in float64 and ships ready
    lhsT tiles; the device program is only DMA + PE + PSUM drains:
      out_i = LA_i^T @ DA + LB_i^T @ DB   per bs-half, where
      LA_i = [Rws_i0 (k 0..63) ; Rws_i1 (k 64..127)],  DA = [d0 ; d1]
      LB_i = [Rws_i2 (k 0..63) ; wt_i (k 64)],         DB = [d2 ; ones]
    (the ones row folds the translation term into the same contraction).
  - Inputs arrive in three DMA chunks ordered so the PE can start on
    (i=0) while the rest is still in flight; outputs leave in two DMA
    chunks so the first half-output overlaps the remaining drains.
  - A dummy-matmul warmup chain keeps the PE continuously busy from
    t~200ns so the pstate ramp reaches full clock before real matmuls.
"""

import numpy as np
from contextlib import ExitStack

import concourse.bass as bass
import concourse.tile as tile
from concourse import bacc, mybir
from concourse.bass_utils import run_bass_kernel_spmd

B, S, P, V = 16, 16, 64, 2562
BS = B * S              # 256
N_CORES = 8
VPAD = 2576             # multiple of 8; per-core N kept even
VC = VPAD // N_CORES    # 322 vertices per core

F32 = mybir.dt.float32
F16 = mybir.dt.float16

# warmup chain tuning (see module docstring)
N_WARM_SMALL = 8
N_WARM_BIG = 7


def _build_kernel():
    nc = bacc.Bacc("TRN2", target_bir_lowering=False, debug=False)

    c1_d = nc.dram_tensor("c1", [128, 256 + VC], F16, kind="ExternalInput").ap()
    c2_d = nc.dram_tensor("c2", [128, 512], F16, kind="ExternalInput").ap()
    c3_d = nc.dram_tensor("c3", [65, VC + 768], F16, kind="ExternalInput").ap()
    out_d = nc.dram_tensor("out", [128, 6 * VC], F16, kind="ExternalOutput").ap()

    with tile.TileContext(nc) as tc, ExitStack() as ctx:
        pool = ctx.enter_context(tc.tile_pool(name="work", bufs=1))
        psum = ctx.enter_context(tc.tile_pool(name="psum", bufs=1, space="PSUM"))

        # ---- PE pstate warmup: keep PE busy from ~200ns so the ramp hits
        # full clock before the real matmuls arrive ----
        wsmall = pool.tile([128, 32], F16, tag="wsmall")
        wbig = pool.tile([128, 512], F16, tag="wbig")
        nc.vector.memset(wsmall[:], 0.0)
        nc.gpsimd.memset(wbig[:], 0.0)
        psw = psum.tile([16, 512], F32)
        for _ in range(N_WARM_SMALL):
            nc.tensor.matmul(psw[:, 0:16], wsmall[:, 0:16], wsmall[:, 16:32],
                             start=True, stop=True)
        for _ in range(N_WARM_BIG):
            nc.tensor.matmul(psw[:], wsmall[:, 0:16], wbig[:],
                             start=True, stop=True)

        # ---- input tiles (three chunks, hot-first) ----
        c1 = pool.tile([128, 256 + VC], F16, tag="c1")   # LA0 | DA
        c2 = pool.tile([128, 512], F16, tag="c2")        # LA1 | LA2
        c3 = pool.tile([65, VC + 768], F16, tag="c3")    # DB | LB0 | LB1 | LB2
        nc.sync.dma_start(out=c1[:], in_=c1_d[:])
        nc.sync.dma_start(out=c2[:], in_=c2_d[:])
        nc.sync.dma_start(out=c3[:], in_=c3_d[:])

        da = c1[:, 256:256 + VC]
        db = c3[:, 0:VC]

        def la(i, h):            # lhsT A-part [128, 128]
            if i == 0:
                return c1[:, h * 128:(h + 1) * 128]
            return c2[:, (i - 1) * 256 + h * 128:(i - 1) * 256 + (h + 1) * 128]

        def lb(i, h):            # lhsT B-part [65, 128]
            base = VC + i * 256 + h * 128
            return c3[:, base:base + 128]

        # ---- matmuls: 6 PSUM groups g=(i,h), A-parts in chunk-arrival
        # order, then B-parts in drain order ----
        groups = [(0, 0), (0, 1), (1, 0), (1, 1), (2, 0), (2, 1)]
        pss = {}
        for g in groups:
            pss[g] = psum.tile([128, VC], F32, name=f"ps{g[0]}{g[1]}")
        for i, h in groups:
            nc.tensor.matmul(pss[(i, h)][:], la(i, h), da, start=True, stop=False)
        for i, h in groups:
            nc.tensor.matmul(pss[(i, h)][:], lb(i, h), db, start=False, stop=True)

        # ---- drains (rotating engines) + two output DMAs ----
        osb = pool.tile([128, 6 * VC], F16, tag="osb")
        drain = [nc.vector.tensor_copy, nc.scalar.copy, nc.vector.tensor_copy,
                 nc.scalar.copy, nc.vector.tensor_copy, nc.scalar.copy]
        for g, (i, h) in enumerate(groups):
            drain[g](osb[:, g * VC:(g + 1) * VC], pss[(i, h)][:])
        nc.sync.dma_start(out=out_d[:, 0:3 * VC], in_=osb[:, 0:3 * VC])
        nc.sync.dma_start(out=out_d[:, 3 * VC:6 * VC], in_=osb[:, 3 * VC:6 * VC])

    nc.compile()
    return nc


_NC_CACHE = None


def _get_nc():
    global _NC_CACHE
    if _NC_CACHE is None:
        _NC_CACHE = _build_kernel()
    return _NC_CACHE


def _prep_inputs(scales, transforms, prototype_weights, prototype_offsets, base_verts):
    """Host-side precompute: rotation matrices, weight folds, shard layout."""
    f = np.float64
    hh = np.float16
    scl = np.asarray(scales, f).reshape(BS, 1)
    tf = np.asarray(transforms, f).reshape(BS, P, 6)
    w = np.asarray(prototype_weights, f).reshape(BS, P)
    t = tf[:, :, 0:3]
    a, b, c = tf[:, :, 3], tf[:, :, 4], tf[:, :, 5]

    ca, sa = np.cos(a), np.sin(a)
    cb, sb = np.cos(b), np.sin(b)
    cc, sc = np.cos(c), np.sin(c)
    R = [
        [cb * cc, -cb * sc, sb],
        [ca * sc + sa * sb * cc, ca * cc - sa * sb * sc, -sa * cb],
        [sa * sc - ca * sb * cc, sa * cc + ca * sb * sc, ca * cb],
    ]
    ws = w * scl                                    # [BS, P]
    wt = np.einsum('sp,spi->is', w, t)              # [3, BS]

    # lhsT blobs (shared across cores): columns are bs, partitions are k
    LA = np.empty((128, 768), f)
    LB = np.empty((65, 768), f)
    for i in range(3):
        LA[0:64, i * 256:(i + 1) * 256] = (R[i][0] * ws).T
        LA[64:128, i * 256:(i + 1) * 256] = (R[i][1] * ws).T
        LB[0:64, i * 256:(i + 1) * 256] = (R[i][2] * ws).T
        LB[64, i * 256:(i + 1) * 256] = wt[i]
    LA = LA.astype(hh)
    LB = LB.astype(hh)

    offp = np.zeros((P, VPAD, 3), np.float32)
    offp[:, :V] = np.asarray(prototype_offsets, np.float32)
    basep = np.zeros((VPAD, 3), np.float32)
    basep[:V] = np.asarray(base_verts, np.float32)
    deformed = (basep[None] + offp).astype(hh)      # [P, VPAD, 3]

    in_maps = []
    for core in range(N_CORES):
        vs = slice(core * VC, (core + 1) * VC)
        d = deformed[:, vs, :]                      # [P, VC, 3]
        c1 = np.empty((128, 256 + VC), hh)
        c1[:, 0:256] = LA[:, 0:256]
        c1[0:64, 256:] = d[:, :, 0]
        c1[64:128, 256:] = d[:, :, 1]
        c3 = np.empty((65, VC + 768), hh)
        c3[0:64, 0:VC] = d[:, :, 2]
        c3[64, 0:VC] = 1.0
        c3[:, VC:] = LB
        in_maps.append({"c1": c1, "c2": LA[:, 256:768].copy(), "c3": c3})
    return in_maps


def kernel(scales, transforms, prototype_weights, prototype_offsets, base_verts):
    nc = _get_nc()
    in_maps = _prep_inputs(
        scales, transforms, prototype_weights, prototype_offsets, base_verts)
    res = run_bass_kernel_spmd(nc, in_maps, list(range(N_CORES)))
    full = np.empty((BS, VPAD, 3), np.float32)
    for core in range(N_CORES):
        planes = res.results[core]["out"].astype(np.float32)  # [128, 6*VC]
        vs = slice(core * VC, (core + 1) * VC)
        for g, (i, h) in enumerate([(0, 0), (0, 1), (1, 0), (1, 1), (2, 0), (2, 1)]):
            full[h * 128:(h + 1) * 128, vs, i] = planes[:, g * VC:(g + 1) * VC]
    return np.ascontiguousarray(full[:, :V, :])


# revision 9
# speedup vs baseline: 1.3585x; 1.0735x over previous
"""Trainium2 Bass kernel for nn_MeshTransformer (8-core SPMD, V-sharded).

Computes, for each of BS=256 (b,s) pairs:
    out[bs, v, i] = sum_{p,j} ws[bs,p] * R[i,j](bs,p) * deformed[p,v,j]
                    + sum_p w[bs,p] * t[bs,p,i]
with R the XYZ-euler rotation, ws = w * scale, deformed = base + offsets.

Mapping:
  - Vertex dim V (2562, padded to 2576) is sharded 8 ways (322/core).
  - The host precomputes every weight product in float64 and ships ready
    lhsT tiles; the device program is only DMA + PE + PSUM drains:
      out_i = LA_i^T @ DA + LB_i^T @ DB   per bs-half, where
      LA_i = [Rws_i0 (k 0..63) ; Rws_i1 (k 64..127)],  DA = [d0 ; d1]
      LB_i = [Rws_i2 (k 0..63) ; wt_i (k 64)],         DB = [d2 ; ones]
    (the ones row folds the translation term into the same contraction).
  - Inputs arrive in three DMA chunks ordered so the PE can start on
    (i=0) while the rest is still in flight; outputs leave in two DMA
    chunks so the first half-output overlaps the remaining drains.
  - A dummy-matmul warmup chain keeps the PE continuously busy from
    t~200ns so the pstate ramp reaches full clock before real matmuls.
"""

import numpy as np
from contextlib import ExitStack

import concourse.bass as bass
import concourse.tile as tile
from concourse import bacc, mybir
from concourse.bass_utils import run_bass_kernel_spmd

B, S, P, V = 16, 16, 64, 2562
BS = B * S              # 256
N_CORES = 8
VPAD = 2576             # multiple of 8; per-core N kept even
VC = VPAD // N_CORES    # 322 vertices per core

F32 = mybir.dt.float32
F16 = mybir.dt.float16

# warmup chain tuning (see module docstring)
N_WARM_SMALL = 24
N_WARM_BIG = 5


def _build_kernel():
    nc = bacc.Bacc("TRN2", target_bir_lowering=False, debug=False)

    c1_d = nc.dram_tensor("c1", [128, 256 + VC], F16, kind="ExternalInput").ap()
    c2_d = nc.dram_tensor("c2", [128, 512], F16, kind="ExternalInput").ap()
    c3_d = nc.dram_tensor("c3", [65, VC + 768], F16, kind="ExternalInput").ap()
    out_d = nc.dram_tensor("out", [128, 6 * VC], F16, kind="ExternalOutput").ap()

    with tile.TileContext(nc) as tc, ExitStack() as ctx:
        pool = ctx.enter_context(tc.tile_pool(name="work", bufs=1))
        psum = ctx.enter_context(tc.tile_pool(name="psum", bufs=1, space="PSUM"))

        # ---- PE pstate warmup: keep PE busy from ~200ns so the ramp hits
        # full clock before the real matmuls arrive ----
        wsmall = pool.tile([128, 32], F16, tag="wsmall")
        wbig = pool.tile([128, 512], F16, tag="wbig")
        nc.vector.memset(wsmall[:], 0.0)
        nc.vector.memset(wbig[:], 0.0)
        psw = psum.tile([16, 512], F32)
        for _ in range(N_WARM_SMALL):
            nc.tensor.matmul(psw[:, 0:16], wsmall[:, 0:16], wsmall[:, 16:32],
                             start=True, stop=True)
        for _ in range(N_WARM_BIG):
            nc.tensor.matmul(psw[:], wsmall[:, 0:16], wbig[:],
                             start=True, stop=True)

        # ---- input tiles (three chunks, hot-first) ----
        c1 = pool.tile([128, 256 + VC], F16, tag="c1")   # LA0 | DA
        c2 = pool.tile([128, 512], F16, tag="c2")        # LA1 | LA2
        c3 = pool.tile([65, VC + 768], F16, tag="c3")    # DB | LB0 | LB1 | LB2
        nc.sync.dma_start(out=c1[:], in_=c1_d[:])
        nc.sync.dma_start(out=c2[:], in_=c2_d[:])
        # c3 rides the Pool engine's SWDGE path: its descriptor-gen runs on
        # the (otherwise idle) Pool engine in parallel with c1/c2's HWDGE.
        nc.gpsimd.dma_start(out=c3[:], in_=c3_d[:])

        da = c1[:, 256:256 + VC]
        db = c3[:, 0:VC]

        def la(i, h):            # lhsT A-part [128, 128]
            if i == 0:
                return c1[:, h * 128:(h + 1) * 128]
            return c2[:, (i - 1) * 256 + h * 128:(i - 1) * 256 + (h + 1) * 128]

        def lb(i, h):            # lhsT B-part [65, 128]
            base = VC + i * 256 + h * 128
            return c3[:, base:base + 128]

        # ---- matmuls: 6 PSUM groups g=(i,h), A-parts in chunk-arrival
        # order, then B-parts in drain order ----
        groups = [(0, 0), (0, 1), (1, 0), (1, 1), (2, 0), (2, 1)]
        pss = {}
        for g in groups:
            pss[g] = psum.tile([128, VC], F32, name=f"ps{g[0]}{g[1]}")
        # group (0,*) completes first (only chunk1+chunk3 needed) so its
        # drains + first output DMA overlap the remaining matmuls
        mm_order = [(0, 0, 'a'), (0, 1, 'a'), (0, 0, 'b'), (0, 1, 'b'),
                    (1, 0, 'a'), (1, 1, 'a'), (2, 0, 'a'), (2, 1, 'a'),
                    (1, 0, 'b'), (1, 1, 'b'), (2, 0, 'b'), (2, 1, 'b')]
        for i, h, part in mm_order:
            if part == 'a':
                nc.tensor.matmul(pss[(i, h)][:], la(i, h), da,
                                 start=True, stop=False)
            else:
                nc.tensor.matmul(pss[(i, h)][:], lb(i, h), db,
                                 start=False, stop=True)

        # ---- drains (rotating engines) + two output DMAs ----
        osb = pool.tile([128, 6 * VC], F16, tag="osb")
        drain = [nc.vector.tensor_copy, nc.scalar.copy, nc.vector.tensor_copy,
                 nc.scalar.copy, nc.vector.tensor_copy, nc.scalar.copy]
        for g, (i, h) in enumerate(groups):
            drain[g](osb[:, g * VC:(g + 1) * VC], pss[(i, h)][:])
        nc.sync.dma_start(out=out_d[:, 0:3 * VC], in_=osb[:, 0:3 * VC])
        nc.sync.dma_start(out=out_d[:, 3 * VC:6 * VC], in_=osb[:, 3 * VC:6 * VC])

    nc.compile()
    return nc


_NC_CACHE = None


def _get_nc():
    global _NC_CACHE
    if _NC_CACHE is None:
        _NC_CACHE = _build_kernel()
    return _NC_CACHE


def _prep_inputs(scales, transforms, prototype_weights, prototype_offsets, base_verts):
    """Host-side precompute: rotation matrices, weight folds, shard layout."""
    f = np.float64
    hh = np.float16
    scl = np.asarray(scales, f).reshape(BS, 1)
    tf = np.asarray(transforms, f).reshape(BS, P, 6)
    w = np.asarray(prototype_weights, f).reshape(BS, P)
    t = tf[:, :, 0:3]
    a, b, c = tf[:, :, 3], tf[:, :, 4], tf[:, :, 5]

    ca, sa = np.cos(a), np.sin(a)
    cb, sb = np.cos(b), np.sin(b)
    cc, sc = np.cos(c), np.sin(c)
    R = [
        [cb * cc, -cb * sc, sb],
        [ca * sc + sa * sb * cc, ca * cc - sa * sb * sc, -sa * cb],
        [sa * sc - ca * sb * cc, sa * cc + ca * sb * sc, ca * cb],
    ]
    ws = w * scl                                    # [BS, P]
    wt = np.einsum('sp,spi->is', w, t)              # [3, BS]

    # lhsT blobs (shared across cores): columns are bs, partitions are k
    LA = np.empty((128, 768), f)
    LB = np.empty((65, 768), f)
    for i in range(3):
        LA[0:64, i * 256:(i + 1) * 256] = (R[i][0] * ws).T
        LA[64:128, i * 256:(i + 1) * 256] = (R[i][1] * ws).T
        LB[0:64, i * 256:(i + 1) * 256] = (R[i][2] * ws).T
        LB[64, i * 256:(i + 1) * 256] = wt[i]
    LA = LA.astype(hh)
    LB = LB.astype(hh)

    offp = np.zeros((P, VPAD, 3), np.float32)
    offp[:, :V] = np.asarray(prototype_offsets, np.float32)
    basep = np.zeros((VPAD, 3), np.float32)
    basep[:V] = np.asarray(base_verts, np.float32)
    deformed = (basep[None] + offp).astype(hh)      # [P, VPAD, 3]

    in_maps = []
    for core in range(N_CORES):
        vs = slice(core * VC, (core + 1) * VC)
        d = deformed[:, vs, :]                      # [P, VC, 3]
        c1 = np.empty((128, 256 + VC), hh)
        c1[:, 0:256] = LA[:, 0:256]
        c1[0:64, 256:] = d[:, :, 0]
        c1[64:128, 256:] = d[:, :, 1]
        c3 = np.empty((65, VC + 768), hh)
        c3[0:64, 0:VC] = d[:, :, 2]
        c3[64, 0:VC] = 1.0
        c3[:, VC:] = LB
        in_maps.append({"c1": c1, "c2": LA[:, 256:768].copy(), "c3": c3})
    return in_maps


def kernel(scales, transforms, prototype_weights, prototype_offsets, base_verts):
    nc = _get_nc()
    in_maps = _prep_inputs(
        scales, transforms, prototype_weights, prototype_offsets, base_verts)
    res = run_bass_kernel_spmd(nc, in_maps, list(range(N_CORES)))
    full = np.empty((BS, VPAD, 3), np.float32)
    for core in range(N_CORES):
        planes = res.results[core]["out"].astype(np.float32)  # [128, 6*VC]
        vs = slice(core * VC, (core + 1) * VC)
        for g, (i, h) in enumerate([(0, 0), (0, 1), (1, 0), (1, 1), (2, 0), (2, 1)]):
            full[h * 128:(h + 1) * 128, vs, i] = planes[:, g * VC:(g + 1) * VC]
    return np.ascontiguousarray(full[:, :V, :])


# revision 11
# speedup vs baseline: 1.4078x; 1.0363x over previous
"""Trainium2 Bass kernel for nn_MeshTransformer (8-core SPMD, V-sharded).

Computes, for each of BS=256 (b,s) pairs:
    out[bs, v, i] = sum_{p,j} ws[bs,p] * R[i,j](bs,p) * deformed[p,v,j]
                    + sum_p w[bs,p] * t[bs,p,i]
with R the XYZ-euler rotation, ws = w * scale, deformed = base + offsets.

Mapping:
  - Vertex dim V (2562, padded to 2576) is sharded 8 ways (322/core).
  - The host precomputes every weight product in float64 and ships ready
    lhsT tiles; the device program is only DMA + PE + PSUM drains:
      out_i = LA_i^T @ DA + LB_i^T @ DB   per bs-half, where
      LA_i = [Rws_i0 (k 0..63) ; Rws_i1 (k 64..127)],  DA = [d0 ; d1]
      LB_i = [Rws_i2 (k 0..63) ; wt_i (k 64)],         DB = [d2 ; ones]
    (the ones row folds the translation term into the same contraction).
  - Inputs arrive in three DMA chunks ordered so the PE can start on
    (i=0) while the rest is still in flight; outputs leave in two DMA
    chunks so the first half-output overlaps the remaining drains.
  - A dummy-matmul warmup chain keeps the PE continuously busy from
    t~200ns so the pstate ramp reaches full clock before real matmuls.
"""

import numpy as np
from contextlib import ExitStack

import concourse.bass as bass
import concourse.tile as tile
from concourse import bacc, mybir
from concourse.bass_utils import run_bass_kernel_spmd

B, S, P, V = 16, 16, 64, 2562
BS = B * S              # 256
N_CORES = 8
VPAD = 2576             # multiple of 8; per-core N kept even
VC = VPAD // N_CORES    # 322 vertices per core

F32 = mybir.dt.float32
F16 = mybir.dt.float16

# warmup chain tuning (see module docstring)
N_WARM_SMALL = 12
N_WARM_MED = 21


def _build_kernel():
    nc = bacc.Bacc("TRN2", target_bir_lowering=False, debug=False)

    c1_d = nc.dram_tensor("c1", [128, 256 + VC], F16, kind="ExternalInput").ap()
    c2_d = nc.dram_tensor("c2", [128, 512], F16, kind="ExternalInput").ap()
    c3_d = nc.dram_tensor("c3", [65, VC + 768], F16, kind="ExternalInput").ap()
    out_d = nc.dram_tensor("out", [128, 6 * VC], F16, kind="ExternalOutput").ap()

    with tile.TileContext(nc) as tc, ExitStack() as ctx:
        pool = ctx.enter_context(tc.tile_pool(name="work", bufs=1))
        psum = ctx.enter_context(tc.tile_pool(name="psum", bufs=1, space="PSUM"))

        # ---- PE pstate warmup: keep PE busy from ~200ns so the ramp hits
        # full clock before the real matmuls arrive ----
        wsmall = pool.tile([128, 32], F16, tag="wsmall")
        wmed = pool.tile([128, 128], F16, tag="wmed")
        nc.gpsimd.memset(wsmall[:], 0.0)
        nc.vector.memset(wmed[:], 0.0)
        psw = psum.tile([16, 512], F32)
        for _ in range(N_WARM_SMALL):
            nc.tensor.matmul(psw[:, 0:16], wsmall[:, 0:16], wsmall[:, 16:32],
                             start=True, stop=True)
        for _ in range(N_WARM_MED):
            nc.tensor.matmul(psw[:, 0:128], wsmall[:, 0:16], wmed[:],
                             start=True, stop=True)

        # ---- input tiles (three chunks, hot-first) ----
        c1 = pool.tile([128, 256 + VC], F16, tag="c1")   # LA0 | DA
        c2 = pool.tile([128, 512], F16, tag="c2")        # LA1 | LA2
        c3 = pool.tile([65, VC + 768], F16, tag="c3")    # DB | LB0 | LB1 | LB2
        nc.sync.dma_start(out=c1[:], in_=c1_d[:])
        nc.sync.dma_start(out=c2[:], in_=c2_d[:])
        # c3 rides the Pool engine's SWDGE path: its descriptor-gen runs on
        # the (otherwise idle) Pool engine in parallel with c1/c2's HWDGE.
        nc.gpsimd.dma_start(out=c3[:], in_=c3_d[:])

        da = c1[:, 256:256 + VC]
        db = c3[:, 0:VC]

        def la(i, h):            # lhsT A-part [128, 128]
            if i == 0:
                return c1[:, h * 128:(h + 1) * 128]
            return c2[:, (i - 1) * 256 + h * 128:(i - 1) * 256 + (h + 1) * 128]

        def lb(i, h):            # lhsT B-part [65, 128]
            base = VC + i * 256 + h * 128
            return c3[:, base:base + 128]

        # ---- matmuls: 6 PSUM groups g=(i,h), A-parts in chunk-arrival
        # order, then B-parts in drain order ----
        groups = [(0, 0), (0, 1), (1, 0), (1, 1), (2, 0), (2, 1)]
        pss = {}
        for g in groups:
            pss[g] = psum.tile([128, VC], F32, name=f"ps{g[0]}{g[1]}")
        # group (0,*) completes first (only chunk1+chunk3 needed) so its
        # drains + first output DMA overlap the remaining matmuls
        mm_order = [(0, 0, 'a'), (0, 1, 'a'), (0, 0, 'b'), (0, 1, 'b'),
                    (1, 0, 'a'), (1, 1, 'a'), (2, 0, 'a'), (2, 1, 'a'),
                    (1, 0, 'b'), (1, 1, 'b'), (2, 0, 'b'), (2, 1, 'b')]
        for i, h, part in mm_order:
            if part == 'a':
                nc.tensor.matmul(pss[(i, h)][:], la(i, h), da,
                                 start=True, stop=False)
            else:
                nc.tensor.matmul(pss[(i, h)][:], lb(i, h), db,
                                 start=False, stop=True)

        # ---- drains (rotating engines) + two output DMAs ----
        osb = pool.tile([128, 6 * VC], F16, tag="osb")
        drain = [nc.vector.tensor_copy, nc.scalar.copy, nc.vector.tensor_copy,
                 nc.scalar.copy, nc.vector.tensor_copy, nc.scalar.copy]
        for g, (i, h) in enumerate(groups):
            drain[g](osb[:, g * VC:(g + 1) * VC], pss[(i, h)][:])
        nc.sync.dma_start(out=out_d[:, 0:3 * VC], in_=osb[:, 0:3 * VC])
        nc.sync.dma_start(out=out_d[:, 3 * VC:6 * VC], in_=osb[:, 3 * VC:6 * VC])

    nc.compile()
    return nc


_NC_CACHE = None


def _get_nc():
    global _NC_CACHE
    if _NC_CACHE is None:
        _NC_CACHE = _build_kernel()
    return _NC_CACHE


def _prep_inputs(scales, transforms, prototype_weights, prototype_offsets, base_verts):
    """Host-side precompute: rotation matrices, weight folds, shard layout."""
    f = np.float64
    hh = np.float16
    scl = np.asarray(scales, f).reshape(BS, 1)
    tf = np.asarray(transforms, f).reshape(BS, P, 6)
    w = np.asarray(prototype_weights, f).reshape(BS, P)
    t = tf[:, :, 0:3]
    a, b, c = tf[:, :, 3], tf[:, :, 4], tf[:, :, 5]

    ca, sa = np.cos(a), np.sin(a)
    cb, sb = np.cos(b), np.sin(b)
    cc, sc = np.cos(c), np.sin(c)
    R = [
        [cb * cc, -cb * sc, sb],
        [ca * sc + sa * sb * cc, ca * cc - sa * sb * sc, -sa * cb],
        [sa * sc - ca * sb * cc, sa * cc + ca * sb * sc, ca * cb],
    ]
    ws = w * scl                                    # [BS, P]
    wt = np.einsum('sp,spi->is', w, t)              # [3, BS]

    # lhsT blobs (shared across cores): columns are bs, partitions are k
    LA = np.empty((128, 768), f)
    LB = np.empty((65, 768), f)
    for i in range(3):
        LA[0:64, i * 256:(i + 1) * 256] = (R[i][0] * ws).T
        LA[64:128, i * 256:(i + 1) * 256] = (R[i][1] * ws).T
        LB[0:64, i * 256:(i + 1) * 256] = (R[i][2] * ws).T
        LB[64, i * 256:(i + 1) * 256] = wt[i]
    LA = LA.astype(hh)
    LB = LB.astype(hh)

    offp = np.zeros((P, VPAD, 3), np.float32)
    offp[:, :V] = np.asarray(prototype_offsets, np.float32)
    basep = np.zeros((VPAD, 3), np.float32)
    basep[:V] = np.asarray(base_verts, np.float32)
    deformed = (basep[None] + offp).astype(hh)      # [P, VPAD, 3]

    in_maps = []
    for core in range(N_CORES):
        vs = slice(core * VC, (core + 1) * VC)
        d = deformed[:, vs, :]                      # [P, VC, 3]
        c1 = np.empty((128, 256 + VC), hh)
        c1[:, 0:256] = LA[:, 0:256]
        c1[0:64, 256:] = d[:, :, 0]
        c1[64:128, 256:] = d[:, :, 1]
        c3 = np.empty((65, VC + 768), hh)
        c3[0:64, 0:VC] = d[:, :, 2]
        c3[64, 0:VC] = 1.0
        c3[:, VC:] = LB
        in_maps.append({"c1": c1, "c2": LA[:, 256:768].copy(), "c3": c3})
    return in_maps


def kernel(scales, transforms, prototype_weights, prototype_offsets, base_verts):
    nc = _get_nc()
    in_maps = _prep_inputs(
        scales, transforms, prototype_weights, prototype_offsets, base_verts)
    res = run_bass_kernel_spmd(nc, in_maps, list(range(N_CORES)))
    full = np.empty((BS, VPAD, 3), np.float32)
    for core in range(N_CORES):
        planes = res.results[core]["out"].astype(np.float32)  # [128, 6*VC]
        vs = slice(core * VC, (core + 1) * VC)
        for g, (i, h) in enumerate([(0, 0), (0, 1), (1, 0), (1, 1), (2, 0), (2, 1)]):
            full[h * 128:(h + 1) * 128, vs, i] = planes[:, g * VC:(g + 1) * VC]
    return np.ascontiguousarray(full[:, :V, :])


# revision 13
# speedup vs baseline: 1.4418x; 1.0241x over previous
"""Trainium2 Bass kernel for nn_MeshTransformer (8-core SPMD, V-sharded).

Computes, for each of BS=256 (b,s) pairs:
    out[bs, v, i] = sum_{p,j} ws[bs,p] * R[i,j](bs,p) * deformed[p,v,j]
                    + sum_p w[bs,p] * t[bs,p,i]
with R the XYZ-euler rotation, ws = w * scale, deformed = base + offsets.

Mapping:
  - Vertex dim V (2562, padded to 2576) is sharded 8 ways (322/core).
  - The host precomputes every weight product in float64 and ships ready
    lhsT tiles; the device program is only DMA + PE + PSUM drains:
      out_i = LA_i^T @ DA + LB_i^T @ DB   per bs-half, where
      LA_i = [Rws_i0 (k 0..63) ; Rws_i1 (k 64..127)],  DA = [d0 ; d1]
      LB_i = [Rws_i2 (k 0..63) ; wt_i (k 64)],         DB = [d2 ; ones]
    (the ones row folds the translation term into the same contraction).
  - Inputs arrive in three DMA chunks ordered so the PE can start on
    (i=0) while the rest is still in flight; outputs leave in two DMA
    chunks so the first half-output overlaps the remaining drains.
  - A dummy-matmul warmup chain keeps the PE continuously busy from
    t~200ns so the pstate ramp reaches full clock before real matmuls.
"""

import numpy as np
from contextlib import ExitStack

import concourse.bass as bass
import concourse.tile as tile
from concourse import bacc, mybir
from concourse.bass_utils import run_bass_kernel_spmd

B, S, P, V = 16, 16, 64, 2562
BS = B * S              # 256
N_CORES = 8
VPAD = 2576             # multiple of 8; per-core N kept even
VC = VPAD // N_CORES    # 322 vertices per core

F32 = mybir.dt.float32
F16 = mybir.dt.float16

# warmup chain tuning (see module docstring)
N_WARM_SMALL = 12
N_WARM_MED = 21


def _build_kernel():
    nc = bacc.Bacc("TRN2", target_bir_lowering=False, debug=False)

    c1_d = nc.dram_tensor("c1", [128, 256 + VC], F16, kind="ExternalInput").ap()
    c2_d = nc.dram_tensor("c2", [128, 512], F16, kind="ExternalInput").ap()
    c3_d = nc.dram_tensor("c3", [65, VC + 768], F16, kind="ExternalInput").ap()
    out_d = nc.dram_tensor("out", [128, 6 * VC], F16, kind="ExternalOutput").ap()

    with tile.TileContext(nc) as tc, ExitStack() as ctx:
        pool = ctx.enter_context(tc.tile_pool(name="work", bufs=1))
        psum = ctx.enter_context(tc.tile_pool(name="psum", bufs=1, space="PSUM"))

        # ---- PE pstate warmup: keep PE busy from ~200ns so the ramp hits
        # full clock before the real matmuls arrive ----
        wsmall = pool.tile([128, 32], F16, tag="wsmall")
        wmed = pool.tile([128, 128], F16, tag="wmed")
        nc.gpsimd.memset(wsmall[:], 0.0)
        nc.vector.memset(wmed[:], 0.0)
        psw = psum.tile([16, 512], F32)
        for _ in range(N_WARM_SMALL):
            nc.tensor.matmul(psw[:, 0:16], wsmall[:, 0:16], wsmall[:, 16:32],
                             start=True, stop=True)
        for _ in range(N_WARM_MED):
            nc.tensor.matmul(psw[:, 0:128], wsmall[:, 0:16], wmed[:],
                             start=True, stop=True)

        # ---- input tiles (three chunks, hot-first) ----
        c1 = pool.tile([128, 256 + VC], F16, tag="c1")   # LA0 | DA
        c2 = pool.tile([128, 512], F16, tag="c2")        # LA1 | LA2
        c3 = pool.tile([65, VC + 768], F16, tag="c3")    # DB | LB0 | LB1 | LB2
        nc.sync.dma_start(out=c1[:], in_=c1_d[:])
        nc.sync.dma_start(out=c2[:], in_=c2_d[:])
        # c3 rides the Pool engine's SWDGE path: its descriptor-gen runs on
        # the (otherwise idle) Pool engine in parallel with c1/c2's HWDGE.
        nc.gpsimd.dma_start(out=c3[:], in_=c3_d[:])

        da = c1[:, 256:256 + VC]
        db = c3[:, 0:VC]

        def la(i, h):            # lhsT A-part [128, 128]
            if i == 0:
                return c1[:, h * 128:(h + 1) * 128]
            return c2[:, (i - 1) * 256 + h * 128:(i - 1) * 256 + (h + 1) * 128]

        def lb(i, h):            # lhsT B-part [65, 128]
            base = VC + i * 256 + h * 128
            return c3[:, base:base + 128]

        # ---- matmuls: 6 PSUM groups g=(i,h), A-parts in chunk-arrival
        # order, then B-parts in drain order ----
        groups = [(0, 0), (0, 1), (1, 0), (1, 1), (2, 0), (2, 1)]
        pss = {}
        for g in groups:
            pss[g] = psum.tile([128, VC], F32, name=f"ps{g[0]}{g[1]}")
        # group (0,*) completes first (only chunk1+chunk3 needed) so its
        # drains + first output DMA overlap the remaining matmuls
        mm_order = [(0, 0, 'a'), (0, 1, 'a'), (0, 0, 'b'), (0, 1, 'b'),
                    (1, 0, 'a'), (1, 1, 'a'), (1, 0, 'b'), (1, 1, 'b'),
                    (2, 0, 'a'), (2, 1, 'a'), (2, 0, 'b'), (2, 1, 'b')]
        for i, h, part in mm_order:
            if part == 'a':
                nc.tensor.matmul(pss[(i, h)][:], la(i, h), da,
                                 start=True, stop=False)
            else:
                nc.tensor.matmul(pss[(i, h)][:], lb(i, h), db,
                                 start=False, stop=True)

        # ---- drains (rotating engines) + two output DMAs ----
        osb = pool.tile([128, 6 * VC], F16, tag="osb")
        drain = [nc.vector.tensor_copy, nc.scalar.copy, nc.vector.tensor_copy,
                 nc.scalar.copy, nc.vector.tensor_copy, nc.scalar.copy]
        for g, (i, h) in enumerate(groups):
            drain[g](osb[:, g * VC:(g + 1) * VC], pss[(i, h)][:])
        nc.sync.dma_start(out=out_d[:, 0:2 * VC], in_=osb[:, 0:2 * VC])
        nc.sync.dma_start(out=out_d[:, 2 * VC:4 * VC], in_=osb[:, 2 * VC:4 * VC])
        nc.sync.dma_start(out=out_d[:, 4 * VC:6 * VC], in_=osb[:, 4 * VC:6 * VC])

    nc.compile()
    return nc


_NC_CACHE = None


def _get_nc():
    global _NC_CACHE
    if _NC_CACHE is None:
        _NC_CACHE = _build_kernel()
    return _NC_CACHE


def _prep_inputs(scales, transforms, prototype_weights, prototype_offsets, base_verts):
    """Host-side precompute: rotation matrices, weight folds, shard layout."""
    f = np.float64
    hh = np.float16
    scl = np.asarray(scales, f).reshape(BS, 1)
    tf = np.asarray(transforms, f).reshape(BS, P, 6)
    w = np.asarray(prototype_weights, f).reshape(BS, P)
    t = tf[:, :, 0:3]
    a, b, c = tf[:, :, 3], tf[:, :, 4], tf[:, :, 5]

    ca, sa = np.cos(a), np.sin(a)
    cb, sb = np.cos(b), np.sin(b)
    cc, sc = np.cos(c), np.sin(c)
    R = [
        [cb * cc, -cb * sc, sb],
        [ca * sc + sa * sb * cc, ca * cc - sa * sb * sc, -sa * cb],
        [sa * sc - ca * sb * cc, sa * cc + ca * sb * sc, ca * cb],
    ]
    ws = w * scl                                    # [BS, P]
    wt = np.einsum('sp,spi->is', w, t)              # [3, BS]

    # lhsT blobs (shared across cores): columns are bs, partitions are k
    LA = np.empty((128, 768), f)
    LB = np.empty((65, 768), f)
    for i in range(3):
        LA[0:64, i * 256:(i + 1) * 256] = (R[i][0] * ws).T
        LA[64:128, i * 256:(i + 1) * 256] = (R[i][1] * ws).T
        LB[0:64, i * 256:(i + 1) * 256] = (R[i][2] * ws).T
        LB[64, i * 256:(i + 1) * 256] = wt[i]
    LA = LA.astype(hh)
    LB = LB.astype(hh)

    offp = np.zeros((P, VPAD, 3), np.float32)
    offp[:, :V] = np.asarray(prototype_offsets, np.float32)
    basep = np.zeros((VPAD, 3), np.float32)
    basep[:V] = np.asarray(base_verts, np.float32)
    deformed = (basep[None] + offp).astype(hh)      # [P, VPAD, 3]

    in_maps = []
    for core in range(N_CORES):
        vs = slice(core * VC, (core + 1) * VC)
        d = deformed[:, vs, :]                      # [P, VC, 3]
        c1 = np.empty((128, 256 + VC), hh)
        c1[:, 0:256] = LA[:, 0:256]
        c1[0:64, 256:] = d[:, :, 0]
        c1[64:128, 256:] = d[:, :, 1]
        c3 = np.empty((65, VC + 768), hh)
        c3[0:64, 0:VC] = d[:, :, 2]
        c3[64, 0:VC] = 1.0
        c3[:, VC:] = LB
        in_maps.append({"c1": c1, "c2": LA[:, 256:768].copy(), "c3": c3})
    return in_maps


def kernel(scales, transforms, prototype_weights, prototype_offsets, base_verts):
    nc = _get_nc()
    in_maps = _prep_inputs(
        scales, transforms, prototype_weights, prototype_offsets, base_verts)
    res = run_bass_kernel_spmd(nc, in_maps, list(range(N_CORES)))
    full = np.empty((BS, VPAD, 3), np.float32)
    for core in range(N_CORES):
        planes = res.results[core]["out"].astype(np.float32)  # [128, 6*VC]
        vs = slice(core * VC, (core + 1) * VC)
        for g, (i, h) in enumerate([(0, 0), (0, 1), (1, 0), (1, 1), (2, 0), (2, 1)]):
            full[h * 128:(h + 1) * 128, vs, i] = planes[:, g * VC:(g + 1) * VC]
    return np.ascontiguousarray(full[:, :V, :])


# revision 15
# speedup vs baseline: 1.4639x; 1.0154x over previous
"""Trainium2 Bass kernel for nn_MeshTransformer (8-core SPMD, V-sharded).

Computes, for each of BS=256 (b,s) pairs:
    out[bs, v, i] = sum_{p,j} ws[bs,p] * R[i,j](bs,p) * deformed[p,v,j]
                    + sum_p w[bs,p] * t[bs,p,i]
with R the XYZ-euler rotation, ws = w * scale, deformed = base + offsets.

Mapping:
  - Vertex dim V (2562, padded to 2576) is sharded 8 ways (322/core).
  - The host precomputes every weight product in float64 and ships ready
    lhsT tiles; the device program is only DMA + PE + PSUM drains:
      out_i = LA_i^T @ DA + LB_i^T @ DB   per bs-half, where
      LA_i = [Rws_i0 (k 0..63) ; Rws_i1 (k 64..127)],  DA = [d0 ; d1]
      LB_i = [Rws_i2 (k 0..63) ; wt_i (k 64)],         DB = [d2 ; ones]
    (the ones row folds the translation term into the same contraction).
  - Inputs arrive in three DMA chunks ordered so the PE can start on
    (i=0) while the rest is still in flight; outputs leave in two DMA
    chunks so the first half-output overlaps the remaining drains.
  - A dummy-matmul warmup chain keeps the PE continuously busy from
    t~200ns so the pstate ramp reaches full clock before real matmuls.
"""

import numpy as np
from contextlib import ExitStack

import concourse.bass as bass
import concourse.tile as tile
from concourse import bacc, mybir
from concourse.bass_utils import run_bass_kernel_spmd

B, S, P, V = 16, 16, 64, 2562
BS = B * S              # 256
N_CORES = 8
VPAD = 2576             # multiple of 8; per-core N kept even
VC = VPAD // N_CORES    # 322 vertices per core

F32 = mybir.dt.float32
F16 = mybir.dt.float16

# warmup chain tuning (see module docstring)
N_WARM_SMALL = 12
N_WARM_MED = 21


def _build_kernel():
    nc = bacc.Bacc("TRN2", target_bir_lowering=False, debug=False)

    c1_d = nc.dram_tensor("c1", [128, 256 + VC], F16, kind="ExternalInput").ap()
    c2_d = nc.dram_tensor("c2", [128, 512], F16, kind="ExternalInput").ap()
    c3_d = nc.dram_tensor("c3", [65, VC + 768], F16, kind="ExternalInput").ap()
    out_d = nc.dram_tensor("out", [128, 6 * VC], F16, kind="ExternalOutput").ap()

    with tile.TileContext(nc) as tc, ExitStack() as ctx:
        pool = ctx.enter_context(tc.tile_pool(name="work", bufs=1))
        psum = ctx.enter_context(tc.tile_pool(name="psum", bufs=1, space="PSUM"))

        # ---- PE pstate warmup: keep PE busy from ~200ns so the ramp hits
        # full clock before the real matmuls arrive ----
        wsmall = pool.tile([128, 32], F16, tag="wsmall")
        wmed = pool.tile([128, 128], F16, tag="wmed")
        nc.gpsimd.memset(wsmall[:], 0.0)
        nc.vector.memset(wmed[:], 0.0)
        psw = psum.tile([16, 512], F32)
        for _ in range(N_WARM_SMALL):
            nc.tensor.matmul(psw[:, 0:16], wsmall[:, 0:16], wsmall[:, 16:32],
                             start=True, stop=True)
        for _ in range(N_WARM_MED):
            nc.tensor.matmul(psw[:, 0:128], wsmall[:, 0:16], wmed[:],
                             start=True, stop=True)

        # ---- input tiles (three chunks, hot-first) ----
        c1 = pool.tile([128, 256 + VC], F16, tag="c1")   # LA0 | DA
        c2 = pool.tile([128, 512], F16, tag="c2")        # LA1 | LA2
        c3 = pool.tile([65, VC + 768], F16, tag="c3")    # DB | LB0 | LB1 | LB2
        nc.sync.dma_start(out=c1[:], in_=c1_d[:])
        nc.sync.dma_start(out=c2[:], in_=c2_d[:])
        # c3 rides the Pool engine's SWDGE path: its descriptor-gen runs on
        # the (otherwise idle) Pool engine in parallel with c1/c2's HWDGE.
        nc.gpsimd.dma_start(out=c3[:], in_=c3_d[:])

        da = c1[:, 256:256 + VC]
        db = c3[:, 0:VC]

        def la(i, h):            # lhsT A-part [128, 128]
            if i == 0:
                return c1[:, h * 128:(h + 1) * 128]
            return c2[:, (i - 1) * 256 + h * 128:(i - 1) * 256 + (h + 1) * 128]

        def lb(i, h):            # lhsT B-part [65, 128]
            base = VC + i * 256 + h * 128
            return c3[:, base:base + 128]

        # ---- matmuls: 6 PSUM groups g=(i,h), A-parts in chunk-arrival
        # order, then B-parts in drain order ----
        groups = [(0, 0), (0, 1), (1, 0), (1, 1), (2, 0), (2, 1)]
        pss = {}
        for g in groups:
            pss[g] = psum.tile([128, VC], F32, name=f"ps{g[0]}{g[1]}")
        # group (0,*) completes first (only chunk1+chunk3 needed) so its
        # drains + first output DMA overlap the remaining matmuls
        mm_order = [(0, 0, 'a'), (0, 1, 'a'), (0, 0, 'b'), (0, 1, 'b'),
                    (1, 0, 'a'), (1, 1, 'a'), (1, 0, 'b'), (1, 1, 'b'),
                    (2, 0, 'a'), (2, 1, 'a'), (2, 0, 'b'), (2, 1, 'b')]
        for i, h, part in mm_order:
            if part == 'a':
                nc.tensor.matmul(pss[(i, h)][:], la(i, h), da,
                                 start=True, stop=False)
            else:
                nc.tensor.matmul(pss[(i, h)][:], lb(i, h), db,
                                 start=False, stop=True)

        # ---- drains (rotating engines) + two output DMAs ----
        osb = pool.tile([128, 6 * VC], F16, tag="osb")
        drain = [nc.vector.tensor_copy, nc.scalar.copy, nc.vector.tensor_copy,
                 nc.scalar.copy, nc.vector.tensor_copy, nc.scalar.copy]
        for g, (i, h) in enumerate(groups):
            drain[g](osb[:, g * VC:(g + 1) * VC], pss[(i, h)][:])
        # two issuing engines: HWDGE (not one SEQ's hold) becomes the cadence
        nc.sync.dma_start(out=out_d[:, 0:2 * VC], in_=osb[:, 0:2 * VC])
        nc.scalar.dma_start(out=out_d[:, 2 * VC:4 * VC], in_=osb[:, 2 * VC:4 * VC])
        nc.sync.dma_start(out=out_d[:, 4 * VC:6 * VC], in_=osb[:, 4 * VC:6 * VC])

    nc.compile()
    return nc


_NC_CACHE = None


def _get_nc():
    global _NC_CACHE
    if _NC_CACHE is None:
        _NC_CACHE = _build_kernel()
    return _NC_CACHE


def _prep_inputs(scales, transforms, prototype_weights, prototype_offsets, base_verts):
    """Host-side precompute: rotation matrices, weight folds, shard layout."""
    f = np.float64
    hh = np.float16
    scl = np.asarray(scales, f).reshape(BS, 1)
    tf = np.asarray(transforms, f).reshape(BS, P, 6)
    w = np.asarray(prototype_weights, f).reshape(BS, P)
    t = tf[:, :, 0:3]
    a, b, c = tf[:, :, 3], tf[:, :, 4], tf[:, :, 5]

    ca, sa = np.cos(a), np.sin(a)
    cb, sb = np.cos(b), np.sin(b)
    cc, sc = np.cos(c), np.sin(c)
    R = [
        [cb * cc, -cb * sc, sb],
        [ca * sc + sa * sb * cc, ca * cc - sa * sb * sc, -sa * cb],
        [sa * sc - ca * sb * cc, sa * cc + ca * sb * sc, ca * cb],
    ]
    ws = w * scl                                    # [BS, P]
    wt = np.einsum('sp,spi->is', w, t)              # [3, BS]

    # lhsT blobs (shared across cores): columns are bs, partitions are k
    LA = np.empty((128, 768), f)
    LB = np.empty((65, 768), f)
    for i in range(3):
        LA[0:64, i * 256:(i + 1) * 256] = (R[i][0] * ws).T
        LA[64:128, i * 256:(i + 1) * 256] = (R[i][1] * ws).T
        LB[0:64, i * 256:(i + 1) * 256] = (R[i][2] * ws).T
        LB[64, i * 256:(i + 1) * 256] = wt[i]
    LA = LA.astype(hh)
    LB = LB.astype(hh)

    offp = np.zeros((P, VPAD, 3), np.float32)
    offp[:, :V] = np.asarray(prototype_offsets, np.float32)
    basep = np.zeros((VPAD, 3), np.float32)
    basep[:V] = np.asarray(base_verts, np.float32)
    deformed = (basep[None] + offp).astype(hh)      # [P, VPAD, 3]

    in_maps = []
    for core in range(N_CORES):
        vs = slice(core * VC, (core + 1) * VC)
        d = deformed[:, vs, :]                      # [P, VC, 3]
        c1 = np.empty((128, 256 + VC), hh)
        c1[:, 0:256] = LA[:, 0:256]
        c1[0:64, 256:] = d[:, :, 0]
        c1[64:128, 256:] = d[:, :, 1]
        c3 = np.empty((65, VC + 768), hh)
        c3[0:64, 0:VC] = d[:, :, 2]
        c3[64, 0:VC] = 1.0
        c3[:, VC:] = LB
        in_maps.append({"c1": c1, "c2": LA[:, 256:768].copy(), "c3": c3})
    return in_maps


def kernel(scales, transforms, prototype_weights, prototype_offsets, base_verts):
    nc = _get_nc()
    in_maps = _prep_inputs(
        scales, transforms, prototype_weights, prototype_offsets, base_verts)
    res = run_bass_kernel_spmd(nc, in_maps, list(range(N_CORES)))
    full = np.empty((BS, VPAD, 3), np.float32)
    for core in range(N_CORES):
        planes = res.results[core]["out"].astype(np.float32)  # [128, 6*VC]
        vs = slice(core * VC, (core + 1) * VC)
        for g, (i, h) in enumerate([(0, 0), (0, 1), (1, 0), (1, 1), (2, 0), (2, 1)]):
            full[h * 128:(h + 1) * 128, vs, i] = planes[:, g * VC:(g + 1) * VC]
    return np.ascontiguousarray(full[:, :V, :])


# revision 16
# speedup vs baseline: 1.4669x; 1.0020x over previous
"""Trainium2 Bass kernel for nn_MeshTransformer (8-core SPMD, V-sharded).

Computes, for each of BS=256 (b,s) pairs:
    out[bs, v, i] = sum_{p,j} ws[bs,p] * R[i,j](bs,p) * deformed[p,v,j]
                    + sum_p w[bs,p] * t[bs,p,i]
with R the XYZ-euler rotation, ws = w * scale, deformed = base + offsets.

Mapping:
  - Vertex dim V (2562, padded to 2576) is sharded 8 ways (322/core).
  - The host precomputes every weight product in float64 and ships ready
    lhsT tiles; the device program is only DMA + PE + PSUM drains:
      out_i = LA_i^T @ DA + LB_i^T @ DB   per bs-half, where
      LA_i = [Rws_i0 (k 0..63) ; Rws_i1 (k 64..127)],  DA = [d0 ; d1]
      LB_i = [Rws_i2 (k 0..63) ; wt_i (k 64)],         DB = [d2 ; ones]
    (the ones row folds the translation term into the same contraction).
  - Inputs arrive in three DMA chunks ordered so the PE can start on
    (i=0) while the rest is still in flight; outputs leave in two DMA
    chunks so the first half-output overlaps the remaining drains.
  - A dummy-matmul warmup chain keeps the PE continuously busy from
    t~200ns so the pstate ramp reaches full clock before real matmuls.
"""

import numpy as np
from contextlib import ExitStack

import concourse.bass as bass
import concourse.tile as tile
from concourse import bacc, mybir
from concourse.bass_utils import run_bass_kernel_spmd

B, S, P, V = 16, 16, 64, 2562
BS = B * S              # 256
N_CORES = 8
VPAD = 2576             # multiple of 8; per-core N kept even
VC = VPAD // N_CORES    # 322 vertices per core

F32 = mybir.dt.float32
F16 = mybir.dt.float16

# warmup chain tuning (see module docstring)
N_WARM_SMALL = 12
N_WARM_MED = 18


def _build_kernel():
    nc = bacc.Bacc("TRN2", target_bir_lowering=False, debug=False)

    c1_d = nc.dram_tensor("c1", [128, 256 + VC], F16, kind="ExternalInput").ap()
    c2_d = nc.dram_tensor("c2", [128, 512], F16, kind="ExternalInput").ap()
    c3_d = nc.dram_tensor("c3", [65, VC + 768], F16, kind="ExternalInput").ap()
    out_d = nc.dram_tensor("out", [128, 6 * VC], F16, kind="ExternalOutput").ap()

    with tile.TileContext(nc) as tc, ExitStack() as ctx:
        pool = ctx.enter_context(tc.tile_pool(name="work", bufs=1))
        psum = ctx.enter_context(tc.tile_pool(name="psum", bufs=1, space="PSUM"))

        # ---- PE pstate warmup: keep PE busy from ~200ns so the ramp hits
        # full clock before the real matmuls arrive ----
        wsmall = pool.tile([128, 32], F16, tag="wsmall")
        wmed = pool.tile([128, 128], F16, tag="wmed")
        nc.gpsimd.memset(wsmall[:], 0.0)
        nc.vector.memset(wmed[:], 0.0)
        psw = psum.tile([16, 512], F32)
        for _ in range(N_WARM_SMALL):
            nc.tensor.matmul(psw[:, 0:16], wsmall[:, 0:16], wsmall[:, 16:32],
                             start=True, stop=True)
        for _ in range(N_WARM_MED):
            nc.tensor.matmul(psw[:, 0:128], wsmall[:, 0:16], wmed[:],
                             start=True, stop=True)

        # ---- input tiles (three chunks, hot-first) ----
        c1 = pool.tile([128, 256 + VC], F16, tag="c1")   # LA0 | DA
        c2 = pool.tile([128, 512], F16, tag="c2")        # LA1 | LA2
        c3 = pool.tile([65, VC + 768], F16, tag="c3")    # DB | LB0 | LB1 | LB2
        nc.sync.dma_start(out=c1[:], in_=c1_d[:])
        nc.sync.dma_start(out=c2[:], in_=c2_d[:])
        # c3 rides the Pool engine's SWDGE path: its descriptor-gen runs on
        # the (otherwise idle) Pool engine in parallel with c1/c2's HWDGE.
        nc.gpsimd.dma_start(out=c3[:], in_=c3_d[:])

        da = c1[:, 256:256 + VC]
        db = c3[:, 0:VC]

        def la(i, h):            # lhsT A-part [128, 128]
            if i == 0:
                return c1[:, h * 128:(h + 1) * 128]
            return c2[:, (i - 1) * 256 + h * 128:(i - 1) * 256 + (h + 1) * 128]

        def lb(i, h):            # lhsT B-part [65, 128]
            base = VC + i * 256 + h * 128
            return c3[:, base:base + 128]

        # ---- matmuls: 6 PSUM groups g=(i,h), A-parts in chunk-arrival
        # order, then B-parts in drain order ----
        groups = [(0, 0), (0, 1), (1, 0), (1, 1), (2, 0), (2, 1)]
        pss = {}
        for g in groups:
            pss[g] = psum.tile([128, VC], F32, name=f"ps{g[0]}{g[1]}")
        # group (0,*) completes first (only chunk1+chunk3 needed) so its
        # drains + first output DMA overlap the remaining matmuls
        mm_order = [(0, 0, 'a'), (0, 1, 'a'), (0, 0, 'b'), (0, 1, 'b'),
                    (1, 0, 'a'), (1, 1, 'a'), (1, 0, 'b'), (1, 1, 'b'),
                    (2, 0, 'a'), (2, 1, 'a'), (2, 0, 'b'), (2, 1, 'b')]
        for i, h, part in mm_order:
            if part == 'a':
                nc.tensor.matmul(pss[(i, h)][:], la(i, h), da,
                                 start=True, stop=False)
            else:
                nc.tensor.matmul(pss[(i, h)][:], lb(i, h), db,
                                 start=False, stop=True)

        # ---- drains (rotating engines) + two output DMAs ----
        osb = pool.tile([128, 6 * VC], F16, tag="osb")
        drain = [nc.vector.tensor_copy, nc.scalar.copy, nc.vector.tensor_copy,
                 nc.scalar.copy, nc.vector.tensor_copy, nc.scalar.copy]
        for g, (i, h) in enumerate(groups):
            drain[g](osb[:, g * VC:(g + 1) * VC], pss[(i, h)][:])
        # two issuing engines: HWDGE (not one SEQ's hold) becomes the cadence
        nc.sync.dma_start(out=out_d[:, 0:2 * VC], in_=osb[:, 0:2 * VC])
        nc.scalar.dma_start(out=out_d[:, 2 * VC:4 * VC], in_=osb[:, 2 * VC:4 * VC])
        nc.sync.dma_start(out=out_d[:, 4 * VC:6 * VC], in_=osb[:, 4 * VC:6 * VC])

    nc.compile()
    return nc


_NC_CACHE = None


def _get_nc():
    global _NC_CACHE
    if _NC_CACHE is None:
        _NC_CACHE = _build_kernel()
    return _NC_CACHE


def _prep_inputs(scales, transforms, prototype_weights, prototype_offsets, base_verts):
    """Host-side precompute: rotation matrices, weight folds, shard layout."""
    f = np.float64
    hh = np.float16
    scl = np.asarray(scales, f).reshape(BS, 1)
    tf = np.asarray(transforms, f).reshape(BS, P, 6)
    w = np.asarray(prototype_weights, f).reshape(BS, P)
    t = tf[:, :, 0:3]
    a, b, c = tf[:, :, 3], tf[:, :, 4], tf[:, :, 5]

    ca, sa = np.cos(a), np.sin(a)
    cb, sb = np.cos(b), np.sin(b)
    cc, sc = np.cos(c), np.sin(c)
    R = [
        [cb * cc, -cb * sc, sb],
        [ca * sc + sa * sb * cc, ca * cc - sa * sb * sc, -sa * cb],
        [sa * sc - ca * sb * cc, sa * cc + ca * sb * sc, ca * cb],
    ]
    ws = w * scl                                    # [BS, P]
    wt = np.einsum('sp,spi->is', w, t)              # [3, BS]

    # lhsT blobs (shared across cores): columns are bs, partitions are k
    LA = np.empty((128, 768), f)
    LB = np.empty((65, 768), f)
    for i in range(3):
        LA[0:64, i * 256:(i + 1) * 256] = (R[i][0] * ws).T
        LA[64:128, i * 256:(i + 1) * 256] = (R[i][1] * ws).T
        LB[0:64, i * 256:(i + 1) * 256] = (R[i][2] * ws).T
        LB[64, i * 256:(i + 1) * 256] = wt[i]
    LA = LA.astype(hh)
    LB = LB.astype(hh)

    offp = np.zeros((P, VPAD, 3), np.float32)
    offp[:, :V] = np.asarray(prototype_offsets, np.float32)
    basep = np.zeros((VPAD, 3), np.float32)
    basep[:V] = np.asarray(base_verts, np.float32)
    deformed = (basep[None] + offp).astype(hh)      # [P, VPAD, 3]

    in_maps = []
    for core in range(N_CORES):
        vs = slice(core * VC, (core + 1) * VC)
        d = deformed[:, vs, :]                      # [P, VC, 3]
        c1 = np.empty((128, 256 + VC), hh)
        c1[:, 0:256] = LA[:, 0:256]
        c1[0:64, 256:] = d[:, :, 0]
        c1[64:128, 256:] = d[:, :, 1]
        c3 = np.empty((65, VC + 768), hh)
        c3[0:64, 0:VC] = d[:, :, 2]
        c3[64, 0:VC] = 1.0
        c3[:, VC:] = LB
        in_maps.append({"c1": c1, "c2": LA[:, 256:768].copy(), "c3": c3})
    return in_maps


def kernel(scales, transforms, prototype_weights, prototype_offsets, base_verts):
    nc = _get_nc()
    in_maps = _prep_inputs(
        scales, transforms, prototype_weights, prototype_offsets, base_verts)
    res = run_bass_kernel_spmd(nc, in_maps, list(range(N_CORES)))
    full = np.empty((BS, VPAD, 3), np.float32)
    for core in range(N_CORES):
        planes = res.results[core]["out"].astype(np.float32)  # [128, 6*VC]
        vs = slice(core * VC, (core + 1) * VC)
        for g, (i, h) in enumerate([(0, 0), (0, 1), (1, 0), (1, 1), (2, 0), (2, 1)]):
            full[h * 128:(h + 1) * 128, vs, i] = planes[:, g * VC:(g + 1) * VC]
    return np.ascontiguousarray(full[:, :V, :])
